# revision 29
# baseline (speedup 1.0000x reference)
"""Trainium2 Bass kernel for nn_DecoderMinLSTMGNN.

Model (per sample): two MinLSTM layers (D=512) over T=4096 steps, residual,
LayerNorm, projection D->1.  B=8 samples are data-parallel across the 8
NeuronCores (one sample per core).

v2 design (vs baseline 344us):
  - all matmuls bf16 (same PE stream rate as f32r, FWL weight loads)
  - ScalarE never leaves the sigmoid act table inside a block: f/i sigmoids
    per group, then ONE batched Reciprocal over the block's [128, 4*TT] den
    tile (dep-gated on all four groups' den adds, so the greedy scheduler
    cannot interleave it early).  ~2 table swaps per (layer, t) block.
  - PSUM: pf(1) + pi(1) + ph(5) + stats(1) banks.  Both LN/stat matmul
    accumulators share one bank (s13 rows at partitions 0..39, s2 rows at
    partitions 64..71) so ph gets 5 buffers and PE never waits on the
    stt consuming ph.
  - elementwise in bf16 (DVE 2x mode); epilogue square on DVE; a = f*r on
    GpSimd (Pool) to offload DVE.
  - scan (VectorE tensor_tensor_scan) stays per (layer, g, t):
    h = a*state - u', u' = (a-1)*zh.
"""

import numpy as np
import ml_dtypes

import concourse.bass as bass
import concourse.mybir as mybir
import concourse.tile as tile
from concourse.bass_utils import run_bass_kernel_spmd

F32 = mybir.dt.float32
BF16 = mybir.dt.bfloat16
AF = mybir.ActivationFunctionType
OP = mybir.AluOpType

NP_BF16 = np.dtype(ml_dtypes.bfloat16)

B, T, D = 8, 4096, 512
OUT = 1
LN_EPS = 1e-5
TT = 512                 # time-tile size
NT = T // TT             # 8 time tiles
G = D // 128             # 4 channel groups
K = D // 128             # 4 contraction chunks

MAX_WAITS = 1
A_ON_GPSIMD = False      # a = f*r on Pool engine (else DVE)


def _split_excess_waits(nc):
    """walrus in this container rejects >1 semaphore wait per instruction
    ("Too many sync wait commands"); move excess waits onto NoOps."""
    for fn in nc.m.functions:
        for bb in fn.blocks:
            new_list = []
            changed = False
            for inst in bb.instructions:
                si = inst.sync_info
                waits = list(si.on_wait) if si is not None and si.on_wait else []
                if len(waits) > MAX_WAITS:
                    changed = True
                    overflow = waits[:-MAX_WAITS]
                    si.on_wait = waits[-MAX_WAITS:]
                    for j in range(0, len(overflow), MAX_WAITS):
                        new_list.append(mybir.InstNoOp(
                            name=f"{inst.name}-waitsplit-{j}",
                            engine=inst.engine,
                            ins=[], outs=[],
                            sync_info=mybir.SyncInfo(
                                on_wait=overflow[j:j + MAX_WAITS], on_update=[]),
                        ))
                new_list.append(inst)
            if changed:
                bb.instructions[:] = new_list
    return nc


def _act_direct(nc, out, in_, func, bias=0.0, scale=1.0):
    """emit InstActivation directly (bass blocks Reciprocal/Rsqrt)."""
    ins = [nc.scalar.lower_ap(in_)]
    for v in (bias, scale, 0.0):
        if isinstance(v, (int, float)):
            ins.append(mybir.ImmediateValue(dtype=mybir.dt.float32, value=float(v)))
        else:
            ins.append(nc.scalar.lower_ap(v))
    return nc.scalar.add_instruction(
        mybir.InstActivation(
            name=nc.get_next_instruction_name(),
            func=func, ins=ins, outs=[nc.scalar.lower_ap(out)]))


def _build_nc():
    nc = bass.Bass()

    xt_d = nc.dram_tensor("xt", [D, T], BF16, kind="ExternalInput")
    wt_d = nc.dram_tensor("wt", [6, D, D], BF16, kind="ExternalInput")
    # fp8 (e4m3) copies for the f/i gate DoubleRow matmuls.
    # x8[p, k, c] = x^T[k*128+p, c]; w8 order (Wf0, Wi0, Wf1, Wi1), x256.
    x8_d = nc.dram_tensor("x8", [128, K, T], mybir.dt.float8e4,
                          kind="ExternalInput")
    w8_d = nc.dram_tensor("w8", [4, 128, K, D], mybir.dt.float8e4,
                          kind="ExternalInput")
    # f/i biases per layer: bias[p, layer, {f,i}, g] = b[g*128+p]
    bias_d = nc.dram_tensor("bias", [128, 2, 2, G], F32, kind="ExternalInput")
    # h-gate bias rows (layer, g) -> [1, 128], matmul'd against a ones row
    brow_d = nc.dram_tensor("brow", [2 * G, 128], BF16, kind="ExternalInput")
    ones_d = nc.dram_tensor("ones", [1, TT], BF16, kind="ExternalInput")
    # stats lhsT per (g,t): col t = 1, col 32+t = wg[g*128:(g+1)*128]
    slt_d = nc.dram_tensor("slt", [G, NT, 128, 40], BF16, kind="ExternalInput")
    # S2 lhsT per t: col t = 1  (written at partition base 64 of stats bank)
    s2l_d = nc.dram_tensor("s2l", [NT, 128, NT], BF16, kind="ExternalInput")
    epi_d = nc.dram_tensor("epi", [NT, 3], F32, kind="ExternalInput")  # [c0, swg/D, eps]
    out_d = nc.dram_tensor("out", [NT, TT], F32, kind="ExternalOutput")

    with tile.TileContext(nc) as tc:
        with (
            tc.tile_pool(name="const", bufs=1) as const,
            tc.tile_pool(name="xtp", bufs=1) as xtp,
            tc.tile_pool(name="fp", bufs=8) as fp,
            tc.tile_pool(name="ip", bufs=4) as ip,
            tc.tile_pool(name="denp", bufs=3) as denp,
            tc.tile_pool(name="rp", bufs=3) as rp,
            tc.tile_pool(name="work", bufs=6) as work,
            tc.tile_pool(name="hpool", bufs=4) as hpool,
            tc.tile_pool(name="epip", bufs=6) as epip,
            tc.tile_pool(name="fin", bufs=1) as fin,
            tc.tile_pool(name="fi_ps", bufs=2, space="PSUM") as fi_ps,
            tc.tile_pool(name="ph_ps", bufs=3, space="PSUM") as ph_ps,
            tc.tile_pool(name="stats_ps", bufs=1, space="PSUM") as stats_ps,
        ):
            # ---- constants ----
            wt_sb = []
            for idx in range(6):
                w = const.tile([128, K, D], BF16, tag=f"wt{idx}")
                nc.sync.dma_start(
                    out=w[:], in_=wt_d[idx].rearrange("(k p) d -> p k d", p=128))
                wt_sb.append(w)
            x8_sb = const.tile([128, K, T], mybir.dt.float8e4, tag="x8")
            for k in range(K):
                for t in range(NT):
                    nc.sync.dma_start(
                        out=x8_sb[:, k, t * TT:(t + 1) * TT],
                        in_=x8_d[:, k, t * TT:(t + 1) * TT])
            w8_sb = []
            for idx in range(4):
                w8 = const.tile([128, K, D], mybir.dt.float8e4, tag=f"w8_{idx}")
                nc.sync.dma_start(out=w8[:], in_=w8_d[idx])
                w8_sb.append(w8)
            # layer-1 f/i rhs: h1 cast to fp8 on GpSimd, same [p, k, c] layout
            h18 = const.tile([128, K, T], mybir.dt.float8e4, tag="h18")
            bias_sb = const.tile([128, 2, 2, G], F32)
            nc.sync.dma_start(out=bias_sb[:], in_=bias_d[:])
            brow_sb = const.tile([1, 2 * G, 128], BF16)
            nc.sync.dma_start(out=brow_sb[:], in_=brow_d[None, :, :])
            ones_sb = const.tile([1, TT], BF16)
            nc.sync.dma_start(out=ones_sb[:], in_=ones_d[:])
            slt_sb = const.tile([128, G, NT, 40], BF16)
            nc.sync.dma_start(
                out=slt_sb[:], in_=slt_d.rearrange("g t p c -> p g t c"))
            s2l_sb = const.tile([128, NT, NT], BF16)
            nc.sync.dma_start(out=s2l_sb[:], in_=s2l_d.rearrange("t p c -> p t c"))
            epi_sb = const.tile([NT, 3], F32)
            nc.sync.dma_start(out=epi_sb[:], in_=epi_d[:])

            # ---- x^T resident tiles, one DMA per (k, t) ----
            xt_sb = [[None] * NT for _ in range(K)]
            for k in range(K):
                for t in range(NT):
                    xx = xtp.tile([128, TT], BF16, tag=f"xt{k}_{t}")
                    nc.sync.dma_start(
                        out=xx[:],
                        in_=xt_d[k * 128:(k + 1) * 128, t * TT:(t + 1) * TT])
                    xt_sb[k][t] = xx

            # persistent stats accumulator, ONE psum bank:
            #   partitions 0..39  : s13 (rows 0-7 = s1 per tile, 32-39 = s3)
            #   partitions 64..71 : s2
            stats = stats_ps.tile([72, TT], F32, tag="stats")
            stats_first = [True]

            h1_sb = [[None] * NT for _ in range(G)]   # layer-1 outputs (bf16)
            h2_sb = [[None] * NT for _ in range(G)]   # layer-2 outputs (bf16)
            # deferred h-gate work for group 3 of the previous tile of each
            # layer: emitted at the START of the layer's next tile so the
            # ph PSUM ring (3 bufs) never self-blocks inside a tile.
            pending = {0: None, 1: None}

            def emit_fi(layer, t, g):
                """fp8 DoubleRow f/i matmuls + sigmoids + den slice"""
                rhs8 = x8_sb if layer == 0 else h18
                tsl = slice(t * TT, (t + 1) * TT)
                DR = mybir.MatmulPerfMode.DoubleRow
                gsl = slice(g * 128, (g + 1) * 128)
                fi = fi_ps.tile([128, 2 * TT], F32, tag="fi")
                pf = fi[:, 0:TT]
                pi = fi[:, TT:2 * TT]
                for q in range(2):
                    nc.tensor.matmul(
                        pf, w8_sb[2 * layer][:, 2 * q:2 * q + 2, gsl],
                        rhs8[:, 2 * q:2 * q + 2, tsl],
                        start=(q == 0), stop=(q == 1), perf_mode=DR)
                for q in range(2):
                    nc.tensor.matmul(
                        pi, w8_sb[2 * layer + 1][:, 2 * q:2 * q + 2, gsl],
                        rhs8[:, 2 * q:2 * q + 2, tsl],
                        start=(q == 0), stop=(q == 1), perf_mode=DR)
                f_sb = fp.tile([128, TT], BF16, tag="f")
                nc.scalar.activation(
                    f_sb[:], pf, AF.Sigmoid, scale=1.0 / 256.0,
                    bias=bias_sb[:, layer, 0, g:g + 1])
                i_sb = ip.tile([128, TT], BF16, tag="i")
                nc.scalar.activation(
                    i_sb[:], pi, AF.Sigmoid, scale=1.0 / 256.0,
                    bias=bias_sb[:, layer, 1, g:g + 1])
                return f_sb, i_sb

            def emit_ph_mms(layer, t, g):
                """h-gate matmuls (+bias row via k=1 matmul) -> PSUM tile"""
                rhs = (xt_sb if layer == 0 else h1_sb)
                widx0 = 3 * layer
                gsl = slice(g * 128, (g + 1) * 128)
                ph = ph_ps.tile([128, TT], F32, tag="ph")
                for k in range(K):
                    nc.tensor.matmul(
                        ph[:], wt_sb[widx0 + 2][:, k, gsl], rhs[k][t][:],
                        start=(k == 0), stop=False)
                nc.tensor.matmul(
                    ph[:], brow_sb[:, layer * G + g, :], ones_sb[:],
                    start=False, stop=True)
                return ph

            def emit_scan(layer, t, g, a_sb, ph):
                """u' + scan for one (layer, t, g); ph is consumed (PSUM)"""
                h_out = (h1_sb if layer == 0 else h2_sb)
                up_sb = work.tile([128, TT], BF16, tag="up")
                nc.vector.scalar_tensor_tensor(
                    up_sb[:], a_sb[:], 1.0, ph[:], OP.subtract, OP.mult)
                h_sb = hpool.tile([128, TT], BF16, tag=f"h{layer}_{g}")
                init = 0.0 if t == 0 else h_out[g][t - 1][:, TT - 1:TT]
                nc.vector.tensor_tensor_scan(
                    h_sb[:], a_sb[:], up_sb[:], init, OP.mult, OP.subtract)
                h_out[g][t] = h_sb
                if layer == 0:
                    # fp8 copy of h1 for layer-1's f/i DoubleRow rhs
                    # (ScalarE Copy: in-table; GpSimd would poison DVE via
                    # SBUF-port contention)
                    nc.scalar.activation(h18[:, g, t * TT:(t + 1) * TT],
                                         h_sb[:], AF.Copy)

            def flush_pending(layer):
                if pending[layer] is None:
                    return
                pt, a_sb = pending[layer]
                pending[layer] = None
                ph = emit_ph_mms(layer, pt, G - 1)
                emit_scan(layer, pt, G - 1, a_sb, ph)

            def layer_tile(layer, t):
                """emit one time-tile of one MinLSTM layer (all 4 groups).
                Emission interleaves fi(g)/ph(g-1) so ScalarE's sigmoid
                chain keeps pace with the PE, and defers group 3's h-gate
                to the next tile of this layer."""
                flush_pending(layer)
                den = denp.tile([128, G * TT], BF16, tag="den")
                f_l, ph_l = [], []
                for g in range(G):
                    f_sb, i_sb = emit_fi(layer, t, g)
                    nc.vector.tensor_add(
                        den[:, g * TT:(g + 1) * TT], f_sb[:], i_sb[:])
                    f_l.append(f_sb)
                    if g >= 1:
                        ph_l.append(emit_ph_mms(layer, t, g - 1))

                # ONE batched reciprocal over all 4 groups' dens
                r_sb = rp.tile([128, G * TT], BF16, tag="r")
                _act_direct(nc, r_sb[:], den[:], AF.Reciprocal)

                a_l = []
                for g in range(G):
                    a_sb = work.tile([128, TT], BF16, tag="a")
                    nc.vector.tensor_mul(
                        a_sb[:], f_l[g][:], r_sb[:, g * TT:(g + 1) * TT])
                    a_l.append(a_sb)
                for g in range(G - 1):
                    emit_scan(layer, t, g, a_l[g], ph_l[g])
                pending[layer] = (t, a_l[G - 1])

            res_sb = [[None] * NT for _ in range(G)]
            sq_sb = [[None] * NT for _ in range(G)]

            def epilogue_compute(t):
                """residual + square for one time tile (GpSimd, off the
                critical path; runs one pipeline stage before the stats
                matmuls consume it)"""
                for g in range(G):
                    res = epip.tile([128, TT], BF16, tag="res")
                    nc.vector.tensor_add(res[:], h2_sb[g][t][:], xt_sb[g][t][:])
                    sq = epip.tile([128, TT], BF16, tag="sq")
                    nc.vector.tensor_mul(sq[:], res[:], res[:])
                    res_sb[g][t] = res
                    sq_sb[g][t] = sq

            def epilogue_mms(t):
                """LN/output stats matmuls for one time tile"""
                for g in range(G):
                    first = stats_first[0]
                    stats_first[0] = False
                    last = (t == NT - 1 and g == G - 1)
                    nc.tensor.matmul(
                        stats[0:40, :], slt_sb[:, g, t, :], res_sb[g][t][:],
                        start=first, stop=last, skip_group_check=True)
                    nc.tensor.matmul(
                        stats[64:72, :], s2l_sb[:, t, :], sq_sb[g][t][:],
                        start=first, stop=last, skip_group_check=True)

            # ---- pipeline: layer-1 lags layer-0 by TWO tiles so the
            # gate-math tail of layer-0 (sig->den->recip->a->stt->scan)
            # never blocks layer-1's matmuls ----
            for t in range(NT):
                layer_tile(0, t)
                if t >= 2:
                    layer_tile(1, t - 2)
                if t >= 3:
                    epilogue_compute(t - 3)
                if t >= 4:
                    epilogue_mms(t - 4)
            flush_pending(0)          # h1(g3, NT-1) before layer(1, NT-1)
            layer_tile(1, NT - 2)
            epilogue_compute(NT - 3)
            epilogue_mms(NT - 4)
            layer_tile(1, NT - 1)
            epilogue_compute(NT - 2)
            epilogue_mms(NT - 3)
            flush_pending(1)          # h2(g3, NT-1)
            epilogue_compute(NT - 1)
            epilogue_mms(NT - 2)
            epilogue_mms(NT - 1)

            # ---- final LN + projection math on [8, 512] ----
            s1 = stats[0:NT, :]
            s3p = stats[32:32 + NT, :]
            s2p = stats[64:64 + NT, :]
            s3_sb = fin.tile([NT, TT], F32, tag="s3f")
            nc.scalar.activation(s3_sb[:], s3p, AF.Copy)
            # nn = (s1 * swg/D) - s3
            nn_sb = fin.tile([NT, TT], F32, tag="nn")
            nc.vector.scalar_tensor_tensor(
                nn_sb[:], s1, epi_sb[:, 1:2], s3_sb[:], OP.mult, OP.subtract)
            # s1sq = (s1/D)^2
            s1sq_sb = fin.tile([NT, TT], F32, tag="s1sq")
            nc.scalar.activation(s1sq_sb[:], s1, AF.Square, scale=1.0 / D)
            # v = s2/D - s1sq
            v_sb = fin.tile([NT, TT], F32, tag="v")
            nc.vector.scalar_tensor_tensor(
                v_sb[:], s2p, 1.0 / D, s1sq_sb[:], OP.mult, OP.subtract)
            # rv = rsqrt(v + eps)  (one act-table switch, at the very end)
            rv_sb = fin.tile([NT, TT], F32, tag="rv")
            _act_direct(nc, rv_sb[:], v_sb[:], AF.Rsqrt, bias=epi_sb[:, 2:3])
            # pr = (nn * -1) * rv = (s3 - mu*swg) * rv
            pr_sb = fin.tile([NT, TT], F32, tag="pr")
            nc.vector.scalar_tensor_tensor(
                pr_sb[:], nn_sb[:], -1.0, rv_sb[:], OP.mult, OP.mult)
            # out = pr + c0
            o_sb = fin.tile([NT, TT], F32, tag="o")
            nc.scalar.activation(o_sb[:], pr_sb[:], AF.Identity,
                                 bias=epi_sb[:, 0:1])
            nc.sync.dma_start(out=out_d[:], in_=o_sb[:])

    _split_excess_waits(nc)
    return nc


_NC_CACHE = None


def _get_nc():
    global _NC_CACHE
    if _NC_CACHE is None:
        _NC_CACHE = _build_nc()
    return _NC_CACHE


def _host_prep(inputs):
    x = np.asarray(inputs["x"], dtype=np.float32)
    Ws = [inputs[n] for n in ("Wf0", "Wi0", "Wh0", "Wf1", "Wi1", "Wh1")]
    bs = [np.asarray(inputs[n], np.float32) for n in
          ("bf0", "bi0", "bh0", "bf1", "bi1", "bh1")]
    wt_all = np.ascontiguousarray(
        np.stack([np.asarray(w, np.float32).T for w in Ws])).astype(NP_BF16)
    # fp8 weights for f/i gates (Wf0, Wi0, Wf1, Wi1), x256 so the smallest
    # entries stay out of the e4m3 subnormal range; the sigmoid activation
    # rescales with scale=1/256.  Layout [p, k, dout] matching the bf16 path.
    NP_FP8 = np.dtype(ml_dtypes.float8_e4m3)
    w8_all = np.zeros((4, 128, K, D), NP_FP8)
    for j, wi in enumerate((0, 1, 3, 4)):   # indices of Wf0,Wi0,Wf1,Wi1 in Ws
        wT = np.asarray(Ws[wi], np.float32).T * 256.0      # [din, dout]
        w8_all[j] = wT.reshape(K, 128, D).transpose(1, 0, 2).astype(NP_FP8)
    # f/i biases: bias[p, layer, {f,i}, g] = b[g*128+p]
    bias_all = np.zeros((128, 2, 2, G), np.float32)
    for layer in range(2):
        for j in range(2):
            bias_all[:, layer, j, :] = bs[3 * layer + j].reshape(G, 128).T
    # h-gate bias rows: brow[layer*G+g, c] = bh[g*128+c]
    brow = np.zeros((2 * G, 128), np.float32)
    for layer in range(2):
        brow[layer * G:(layer + 1) * G] = bs[3 * layer + 2].reshape(G, 128)
    brow = brow.astype(NP_BF16)
    ones = np.ones((1, TT), NP_BF16)

    w_out = np.asarray(inputs["W_out"], np.float32).reshape(D)
    ln_g = np.asarray(inputs["ln_g"], np.float32)
    ln_b = np.asarray(inputs["ln_b"], np.float32)
    b_out = np.asarray(inputs["b_out"], np.float32).reshape(())
    wg = w_out * ln_g
    c0 = float(np.dot(w_out, ln_b) + b_out)
    swg = float(wg.sum())

    slt = np.zeros((G, NT, 128, 40), np.float32)
    for g in range(G):
        for t in range(NT):
            slt[g, t, :, t] = 1.0
            slt[g, t, :, 32 + t] = wg[g * 128:(g + 1) * 128]
    slt = slt.astype(NP_BF16)
    s2l = np.zeros((NT, 128, NT), np.float32)
    for t in range(NT):
        s2l[t, :, t] = 1.0
    s2l = s2l.astype(NP_BF16)
    epi = np.zeros((NT, 3), np.float32)
    epi[:, 0] = c0
    epi[:, 1] = swg / D
    epi[:, 2] = LN_EPS
    return x, wt_all, w8_all, bias_all, brow, ones, slt, s2l, epi


def _in_maps(inputs):
    x, wt_all, w8_all, bias_all, brow, ones, slt, s2l, epi = _host_prep(inputs)
    NP_FP8 = np.dtype(ml_dtypes.float8_e4m3)
    maps = []
    for b in range(B):
        xt = np.ascontiguousarray(x[b].T)                  # [D, T] f32
        x8 = xt.reshape(K, 128, T).transpose(1, 0, 2).astype(NP_FP8)
        maps.append({
            "xt": xt.astype(NP_BF16), "x8": x8,
            "wt": wt_all, "w8": w8_all, "bias": bias_all, "brow": brow,
            "ones": ones, "slt": slt, "s2l": s2l, "epi": epi,
        })
    return maps


def kernel(**inputs):
    nc = _get_nc()
    res = run_bass_kernel_spmd(nc, _in_maps(inputs), list(range(B)))
    out = np.stack([res.results[b]["out"].reshape(T, OUT) for b in range(B)])
    return out.astype(np.float32)


def kernel_traced(**inputs):
    """same as kernel() but returns (output, BassKernelResults) with timing"""
    nc = _get_nc()
    res = run_bass_kernel_spmd(nc, _in_maps(inputs), list(range(B)), trace=True)
    out = np.stack([res.results[b]["out"].reshape(T, OUT) for b in range(B)])
    return out.astype(np.float32), res


# revision 39
# speedup vs baseline: 1.0002x; 1.0002x over previous
"""Trainium2 Bass kernel for nn_DecoderMinLSTMGNN.

Model (per sample): two MinLSTM layers (D=512) over T=4096 steps, residual,
LayerNorm, projection D->1.  B=8 samples are data-parallel across the 8
NeuronCores (one sample per core).

v2 design (vs baseline 344us):
  - all matmuls bf16 (same PE stream rate as f32r, FWL weight loads)
  - ScalarE never leaves the sigmoid act table inside a block: f/i sigmoids
    per group, then ONE batched Reciprocal over the block's [128, 4*TT] den
    tile (dep-gated on all four groups' den adds, so the greedy scheduler
    cannot interleave it early).  ~2 table swaps per (layer, t) block.
  - PSUM: pf(1) + pi(1) + ph(5) + stats(1) banks.  Both LN/stat matmul
    accumulators share one bank (s13 rows at partitions 0..39, s2 rows at
    partitions 64..71) so ph gets 5 buffers and PE never waits on the
    stt consuming ph.
  - elementwise in bf16 (DVE 2x mode); epilogue square on DVE; a = f*r on
    GpSimd (Pool) to offload DVE.
  - scan (VectorE tensor_tensor_scan) stays per (layer, g, t):
    h = a*state - u', u' = (a-1)*zh.
"""

import numpy as np
import ml_dtypes

import concourse.bass as bass
import concourse.mybir as mybir
import concourse.tile as tile
from concourse.bass_utils import run_bass_kernel_spmd

F32 = mybir.dt.float32
BF16 = mybir.dt.bfloat16
AF = mybir.ActivationFunctionType
OP = mybir.AluOpType

NP_BF16 = np.dtype(ml_dtypes.bfloat16)

B, T, D = 8, 4096, 512
OUT = 1
LN_EPS = 1e-5
TT = 512                 # time-tile size
NT = T // TT             # 8 time tiles
G = D // 128             # 4 channel groups
K = D // 128             # 4 contraction chunks

MAX_WAITS = 1
A_ON_GPSIMD = False      # a = f*r on Pool engine (else DVE)


def _split_excess_waits(nc):
    """walrus in this container rejects >1 semaphore wait per instruction
    ("Too many sync wait commands"); move excess waits onto NoOps."""
    for fn in nc.m.functions:
        for bb in fn.blocks:
            new_list = []
            changed = False
            for inst in bb.instructions:
                si = inst.sync_info
                waits = list(si.on_wait) if si is not None and si.on_wait else []
                if len(waits) > MAX_WAITS:
                    changed = True
                    overflow = waits[:-MAX_WAITS]
                    si.on_wait = waits[-MAX_WAITS:]
                    for j in range(0, len(overflow), MAX_WAITS):
                        new_list.append(mybir.InstNoOp(
                            name=f"{inst.name}-waitsplit-{j}",
                            engine=inst.engine,
                            ins=[], outs=[],
                            sync_info=mybir.SyncInfo(
                                on_wait=overflow[j:j + MAX_WAITS], on_update=[]),
                        ))
                new_list.append(inst)
            if changed:
                bb.instructions[:] = new_list
    return nc


def _act_direct(nc, out, in_, func, bias=0.0, scale=1.0):
    """emit InstActivation directly (bass blocks Reciprocal/Rsqrt)."""
    ins = [nc.scalar.lower_ap(in_)]
    for v in (bias, scale, 0.0):
        if isinstance(v, (int, float)):
            ins.append(mybir.ImmediateValue(dtype=mybir.dt.float32, value=float(v)))
        else:
            ins.append(nc.scalar.lower_ap(v))
    return nc.scalar.add_instruction(
        mybir.InstActivation(
            name=nc.get_next_instruction_name(),
            func=func, ins=ins, outs=[nc.scalar.lower_ap(out)]))


def _build_nc():
    nc = bass.Bass()

    xt_d = nc.dram_tensor("xt", [D, T], BF16, kind="ExternalInput")
    wt_d = nc.dram_tensor("wt", [6, D, D], BF16, kind="ExternalInput")
    # fp8 (e4m3) copies for the f/i gate DoubleRow matmuls.
    # x8[p, k, c] = x^T[k*128+p, c]; w8 order (Wf0, Wi0, Wf1, Wi1), x256.
    x8_d = nc.dram_tensor("x8", [128, K, T], mybir.dt.float8e4,
                          kind="ExternalInput")
    w8_d = nc.dram_tensor("w8", [4, 128, K, D], mybir.dt.float8e4,
                          kind="ExternalInput")
    # f/i biases per layer: bias[p, layer, {f,i}, g] = b[g*128+p]
    bias_d = nc.dram_tensor("bias", [128, 2, 2, G], F32, kind="ExternalInput")
    # h-gate bias rows (layer, g) -> [1, 128], matmul'd against a ones row
    brow_d = nc.dram_tensor("brow", [2 * G, 128], BF16, kind="ExternalInput")
    # f/i bias rows x256 (layer, {f,i}, g) -> [1, 128], matmul'd vs ones;
    # lets one [128, 2*TT] sigmoid act (scale=1/256) cover both gates
    fib_d = nc.dram_tensor("fib", [2 * 2 * G, 128], BF16, kind="ExternalInput")
    ones_d = nc.dram_tensor("ones", [1, TT], BF16, kind="ExternalInput")
    # stats lhsT per (g,t): col t = 1, col 32+t = wg[g*128:(g+1)*128]
    slt_d = nc.dram_tensor("slt", [G, NT, 128, 40], BF16, kind="ExternalInput")
    # S2 lhsT per t: col t = 1  (written at partition base 64 of stats bank)
    s2l_d = nc.dram_tensor("s2l", [NT, 128, NT], BF16, kind="ExternalInput")
    epi_d = nc.dram_tensor("epi", [NT, 3], F32, kind="ExternalInput")  # [c0, swg/D, eps]
    out_d = nc.dram_tensor("out", [NT, TT], F32, kind="ExternalOutput")

    with tile.TileContext(nc) as tc:
        with (
            tc.tile_pool(name="const", bufs=1) as const,
            tc.tile_pool(name="xtp", bufs=1) as xtp,
            tc.tile_pool(name="fp", bufs=4) as fp,
            tc.tile_pool(name="denp", bufs=3) as denp,
            tc.tile_pool(name="rp", bufs=3) as rp,
            tc.tile_pool(name="work", bufs=6) as work,
            tc.tile_pool(name="hpool", bufs=4) as hpool,
            tc.tile_pool(name="epip", bufs=6) as epip,
            tc.tile_pool(name="fin", bufs=1) as fin,
            tc.tile_pool(name="fi_ps", bufs=1, space="PSUM") as fi_ps,
            tc.tile_pool(name="ph_ps", bufs=5, space="PSUM") as ph_ps,
            tc.tile_pool(name="stats_ps", bufs=1, space="PSUM") as stats_ps,
        ):
            # ---- constants ----
            wt_sb = []
            for idx in range(6):
                w = const.tile([128, K, D], BF16, tag=f"wt{idx}")
                nc.sync.dma_start(
                    out=w[:], in_=wt_d[idx].rearrange("(k p) d -> p k d", p=128))
                wt_sb.append(w)
            x8_sb = const.tile([128, K, T], mybir.dt.float8e4, tag="x8")
            for k in range(K):
                for t in range(NT):
                    nc.sync.dma_start(
                        out=x8_sb[:, k, t * TT:(t + 1) * TT],
                        in_=x8_d[:, k, t * TT:(t + 1) * TT])
            w8_sb = []
            for idx in range(4):
                w8 = const.tile([128, K, D], mybir.dt.float8e4, tag=f"w8_{idx}")
                nc.sync.dma_start(out=w8[:], in_=w8_d[idx])
                w8_sb.append(w8)
            # layer-1 f/i rhs: h1 cast to fp8 on GpSimd, same [p, k, c] layout
            h18 = const.tile([128, K, T], mybir.dt.float8e4, tag="h18")
            bias_sb = const.tile([128, 2, 2, G], F32)
            nc.sync.dma_start(out=bias_sb[:], in_=bias_d[:])
            brow_sb = const.tile([1, 2 * G, 128], BF16)
            nc.sync.dma_start(out=brow_sb[:], in_=brow_d[None, :, :])
            fib_sb = const.tile([1, 2 * 2 * G, 128], BF16)
            nc.sync.dma_start(out=fib_sb[:], in_=fib_d[None, :, :])
            ones_sb = const.tile([1, TT], BF16)
            nc.sync.dma_start(out=ones_sb[:], in_=ones_d[:])
            slt_sb = const.tile([128, G, NT, 40], BF16)
            nc.sync.dma_start(
                out=slt_sb[:], in_=slt_d.rearrange("g t p c -> p g t c"))
            s2l_sb = const.tile([128, NT, NT], BF16)
            nc.sync.dma_start(out=s2l_sb[:], in_=s2l_d.rearrange("t p c -> p t c"))
            epi_sb = const.tile([NT, 3], F32)
            nc.sync.dma_start(out=epi_sb[:], in_=epi_d[:])

            # ---- x^T resident tiles, one DMA per (k, t) ----
            xt_sb = [[None] * NT for _ in range(K)]
            for k in range(K):
                for t in range(NT):
                    xx = xtp.tile([128, TT], BF16, tag=f"xt{k}_{t}")
                    nc.sync.dma_start(
                        out=xx[:],
                        in_=xt_d[k * 128:(k + 1) * 128, t * TT:(t + 1) * TT])
                    xt_sb[k][t] = xx

            # persistent stats accumulator, ONE psum bank:
            #   partitions 0..39  : s13 (rows 0-7 = s1 per tile, 32-39 = s3)
            #   partitions 64..71 : s2
            stats = stats_ps.tile([72, TT], F32, tag="stats")
            stats_first = [True]

            h1_sb = [[None] * NT for _ in range(G)]   # layer-1 outputs (bf16)
            h2_sb = [[None] * NT for _ in range(G)]   # layer-2 outputs (bf16)
            # deferred h-gate work for group 3 of the previous tile of each
            # layer: emitted at the START of the layer's next tile so the
            # ph PSUM ring (3 bufs) never self-blocks inside a tile.
            pending = {0: None, 1: None}

            def emit_fi(layer, t, g):
                """fp8 DoubleRow f/i matmuls (+x256 bias rows) and ONE
                batched sigmoid over [128, 2*TT] (scale=1/256)"""
                rhs8 = x8_sb if layer == 0 else h18
                tsl = slice(t * TT, (t + 1) * TT)
                DR = mybir.MatmulPerfMode.DoubleRow
                gsl = slice(g * 128, (g + 1) * 128)
                fi = fi_ps.tile([128, 2 * TT], F32, tag="fi")
                for j, half in ((0, fi[:, 0:TT]), (1, fi[:, TT:2 * TT])):
                    for q in range(2):
                        nc.tensor.matmul(
                            half, w8_sb[2 * layer + j][:, 2 * q:2 * q + 2, gsl],
                            rhs8[:, 2 * q:2 * q + 2, tsl],
                            start=(q == 0), stop=False, perf_mode=DR)
                    nc.tensor.matmul(
                        half, fib_sb[:, (layer * 2 + j) * G + g, :], ones_sb[:],
                        start=False, stop=True)
                fiout = fp.tile([128, 2 * TT], BF16, tag="f")
                nc.scalar.activation(
                    fiout[:], fi[:], AF.Sigmoid, scale=1.0 / 256.0)
                return fiout[:, 0:TT], fiout[:, TT:2 * TT]

            def emit_ph_mms(layer, t, g):
                """h-gate matmuls (+bias row via k=1 matmul) -> PSUM tile"""
                rhs = (xt_sb if layer == 0 else h1_sb)
                widx0 = 3 * layer
                gsl = slice(g * 128, (g + 1) * 128)
                ph = ph_ps.tile([128, TT], F32, tag="ph")
                for k in range(K):
                    nc.tensor.matmul(
                        ph[:], wt_sb[widx0 + 2][:, k, gsl], rhs[k][t][:],
                        start=(k == 0), stop=False)
                nc.tensor.matmul(
                    ph[:], brow_sb[:, layer * G + g, :], ones_sb[:],
                    start=False, stop=True)
                return ph

            def emit_scan(layer, t, g, a_sb, ph):
                """u' + scan for one (layer, t, g); ph is consumed (PSUM)"""
                h_out = (h1_sb if layer == 0 else h2_sb)
                up_sb = work.tile([128, TT], BF16, tag="up")
                nc.vector.scalar_tensor_tensor(
                    up_sb[:], a_sb[:], 1.0, ph[:], OP.subtract, OP.mult)
                h_sb = hpool.tile([128, TT], BF16, tag=f"h{layer}_{g}")
                init = 0.0 if t == 0 else h_out[g][t - 1][:, TT - 1:TT]
                nc.vector.tensor_tensor_scan(
                    h_sb[:], a_sb[:], up_sb[:], init, OP.mult, OP.subtract)
                h_out[g][t] = h_sb
                if layer == 0:
                    # fp8 copy of h1 for layer-1's f/i DoubleRow rhs
                    # (ScalarE Copy: in-table; GpSimd would poison DVE via
                    # SBUF-port contention)
                    nc.scalar.activation(h18[:, g, t * TT:(t + 1) * TT],
                                         h_sb[:], AF.Copy)

            def flush_pending(layer):
                if pending[layer] is None:
                    return
                pt, a_sb = pending[layer]
                pending[layer] = None
                ph = emit_ph_mms(layer, pt, G - 1)
                emit_scan(layer, pt, G - 1, a_sb, ph)

            def layer_tile(layer, t):
                """emit one time-tile of one MinLSTM layer (all 4 groups).
                Emission interleaves fi(g)/ph(g-1) so ScalarE's sigmoid
                chain keeps pace with the PE, and defers group 3's h-gate
                to the next tile of this layer."""
                flush_pending(layer)
                den = denp.tile([128, G * TT], BF16, tag="den")
                f_l, ph_l = [], []
                for g in range(G):
                    f_ap, i_ap = emit_fi(layer, t, g)
                    nc.vector.tensor_add(
                        den[:, g * TT:(g + 1) * TT], f_ap, i_ap)
                    f_l.append(f_ap)
                    if g >= 1:
                        ph_l.append(emit_ph_mms(layer, t, g - 1))

                # ONE batched reciprocal over all 4 groups' dens
                r_sb = rp.tile([128, G * TT], BF16, tag="r")
                _act_direct(nc, r_sb[:], den[:], AF.Reciprocal)

                a_l = []
                for g in range(G):
                    a_sb = work.tile([128, TT], BF16, tag="a")
                    nc.vector.tensor_mul(
                        a_sb[:], f_l[g], r_sb[:, g * TT:(g + 1) * TT])
                    a_l.append(a_sb)
                for g in range(G - 1):
                    emit_scan(layer, t, g, a_l[g], ph_l[g])
                pending[layer] = (t, a_l[G - 1])

            res_sb = [[None] * NT for _ in range(G)]
            sq_sb = [[None] * NT for _ in range(G)]

            def epilogue_compute(t):
                """residual + square for one time tile (GpSimd, off the
                critical path; runs one pipeline stage before the stats
                matmuls consume it)"""
                for g in range(G):
                    res = epip.tile([128, TT], BF16, tag="res")
                    nc.vector.tensor_add(res[:], h2_sb[g][t][:], xt_sb[g][t][:])
                    sq = epip.tile([128, TT], BF16, tag="sq")
                    nc.vector.tensor_mul(sq[:], res[:], res[:])
                    res_sb[g][t] = res
                    sq_sb[g][t] = sq

            def epilogue_mms(t):
                """LN/output stats matmuls for one time tile"""
                for g in range(G):
                    first = stats_first[0]
                    stats_first[0] = False
                    last = (t == NT - 1 and g == G - 1)
                    nc.tensor.matmul(
                        stats[0:40, :], slt_sb[:, g, t, :], res_sb[g][t][:],
                        start=first, stop=last, skip_group_check=True)
                    nc.tensor.matmul(
                        stats[64:72, :], s2l_sb[:, t, :], sq_sb[g][t][:],
                        start=first, stop=last, skip_group_check=True)

            # ---- pipeline: layer-1 lags layer-0 by TWO tiles so the
            # gate-math tail of layer-0 (sig->den->recip->a->stt->scan)
            # never blocks layer-1's matmuls ----
            for t in range(NT):
                layer_tile(0, t)
                if t >= 2:
                    layer_tile(1, t - 2)
                if t >= 3:
                    epilogue_compute(t - 3)
                if t >= 4:
                    epilogue_mms(t - 4)
            flush_pending(0)          # h1(g3, NT-1) before layer(1, NT-1)
            layer_tile(1, NT - 2)
            epilogue_compute(NT - 3)
            epilogue_mms(NT - 4)
            layer_tile(1, NT - 1)
            epilogue_compute(NT - 2)
            epilogue_mms(NT - 3)
            flush_pending(1)          # h2(g3, NT-1)
            epilogue_compute(NT - 1)
            epilogue_mms(NT - 2)
            epilogue_mms(NT - 1)

            # ---- final LN + projection math on [8, 512] ----
            s1 = stats[0:NT, :]
            s3p = stats[32:32 + NT, :]
            s2p = stats[64:64 + NT, :]
            s3_sb = fin.tile([NT, TT], F32, tag="s3f")
            nc.scalar.activation(s3_sb[:], s3p, AF.Copy)
            # nn = (s1 * swg/D) - s3
            nn_sb = fin.tile([NT, TT], F32, tag="nn")
            nc.vector.scalar_tensor_tensor(
                nn_sb[:], s1, epi_sb[:, 1:2], s3_sb[:], OP.mult, OP.subtract)
            # s1sq = (s1/D)^2
            s1sq_sb = fin.tile([NT, TT], F32, tag="s1sq")
            nc.scalar.activation(s1sq_sb[:], s1, AF.Square, scale=1.0 / D)
            # v = s2/D - s1sq
            v_sb = fin.tile([NT, TT], F32, tag="v")
            nc.vector.scalar_tensor_tensor(
                v_sb[:], s2p, 1.0 / D, s1sq_sb[:], OP.mult, OP.subtract)
            # rv = rsqrt(v + eps)  (one act-table switch, at the very end)
            rv_sb = fin.tile([NT, TT], F32, tag="rv")
            _act_direct(nc, rv_sb[:], v_sb[:], AF.Rsqrt, bias=epi_sb[:, 2:3])
            # pr = (nn * -1) * rv = (s3 - mu*swg) * rv
            pr_sb = fin.tile([NT, TT], F32, tag="pr")
            nc.vector.scalar_tensor_tensor(
                pr_sb[:], nn_sb[:], -1.0, rv_sb[:], OP.mult, OP.mult)
            # out = pr + c0
            o_sb = fin.tile([NT, TT], F32, tag="o")
            nc.scalar.activation(o_sb[:], pr_sb[:], AF.Identity,
                                 bias=epi_sb[:, 0:1])
            nc.sync.dma_start(out=out_d[:], in_=o_sb[:])

    _split_excess_waits(nc)
    return nc


_NC_CACHE = None


def _get_nc():
    global _NC_CACHE
    if _NC_CACHE is None:
        _NC_CACHE = _build_nc()
    return _NC_CACHE


def _host_prep(inputs):
    x = np.asarray(inputs["x"], dtype=np.float32)
    Ws = [inputs[n] for n in ("Wf0", "Wi0", "Wh0", "Wf1", "Wi1", "Wh1")]
    bs = [np.asarray(inputs[n], np.float32) for n in
          ("bf0", "bi0", "bh0", "bf1", "bi1", "bh1")]
    wt_all = np.ascontiguousarray(
        np.stack([np.asarray(w, np.float32).T for w in Ws])).astype(NP_BF16)
    # fp8 weights for f/i gates (Wf0, Wi0, Wf1, Wi1), x256 so the smallest
    # entries stay out of the e4m3 subnormal range; the sigmoid activation
    # rescales with scale=1/256.  Layout [p, k, dout] matching the bf16 path.
    NP_FP8 = np.dtype(ml_dtypes.float8_e4m3)
    w8_all = np.zeros((4, 128, K, D), NP_FP8)
    for j, wi in enumerate((0, 1, 3, 4)):   # indices of Wf0,Wi0,Wf1,Wi1 in Ws
        wT = np.asarray(Ws[wi], np.float32).T * 256.0      # [din, dout]
        w8_all[j] = wT.reshape(K, 128, D).transpose(1, 0, 2).astype(NP_FP8)
    # f/i biases: bias[p, layer, {f,i}, g] = b[g*128+p]
    bias_all = np.zeros((128, 2, 2, G), np.float32)
    for layer in range(2):
        for j in range(2):
            bias_all[:, layer, j, :] = bs[3 * layer + j].reshape(G, 128).T
    # h-gate bias rows: brow[layer*G+g, c] = bh[g*128+c]
    brow = np.zeros((2 * G, 128), np.float32)
    for layer in range(2):
        brow[layer * G:(layer + 1) * G] = bs[3 * layer + 2].reshape(G, 128)
    brow = brow.astype(NP_BF16)
    # f/i bias rows x256: fib[(layer*2+j)*G+g, c] = b[g*128+c] * 256
    fib = np.zeros((2 * 2 * G, 128), np.float32)
    for layer in range(2):
        for j in range(2):
            fib[(layer * 2 + j) * G:(layer * 2 + j + 1) * G] = \
                bs[3 * layer + j].reshape(G, 128) * 256.0
    fib = fib.astype(NP_BF16)
    ones = np.ones((1, TT), NP_BF16)

    w_out = np.asarray(inputs["W_out"], np.float32).reshape(D)
    ln_g = np.asarray(inputs["ln_g"], np.float32)
    ln_b = np.asarray(inputs["ln_b"], np.float32)
    b_out = np.asarray(inputs["b_out"], np.float32).reshape(())
    wg = w_out * ln_g
    c0 = float(np.dot(w_out, ln_b) + b_out)
    swg = float(wg.sum())

    slt = np.zeros((G, NT, 128, 40), np.float32)
    for g in range(G):
        for t in range(NT):
            slt[g, t, :, t] = 1.0
            slt[g, t, :, 32 + t] = wg[g * 128:(g + 1) * 128]
    slt = slt.astype(NP_BF16)
    s2l = np.zeros((NT, 128, NT), np.float32)
    for t in range(NT):
        s2l[t, :, t] = 1.0
    s2l = s2l.astype(NP_BF16)
    epi = np.zeros((NT, 3), np.float32)
    epi[:, 0] = c0
    epi[:, 1] = swg / D
    epi[:, 2] = LN_EPS
    return x, wt_all, w8_all, bias_all, brow, fib, ones, slt, s2l, epi


def _in_maps(inputs):
    (x, wt_all, w8_all, bias_all, brow, fib, ones, slt, s2l,
     epi) = _host_prep(inputs)
    NP_FP8 = np.dtype(ml_dtypes.float8_e4m3)
    maps = []
    for b in range(B):
        xt = np.ascontiguousarray(x[b].T)                  # [D, T] f32
        x8 = xt.reshape(K, 128, T).transpose(1, 0, 2).astype(NP_FP8)
        maps.append({
            "xt": xt.astype(NP_BF16), "x8": x8,
            "wt": wt_all, "w8": w8_all, "bias": bias_all, "brow": brow,
            "fib": fib, "ones": ones, "slt": slt, "s2l": s2l, "epi": epi,
        })
    return maps


def kernel(**inputs):
    nc = _get_nc()
    res = run_bass_kernel_spmd(nc, _in_maps(inputs), list(range(B)))
    out = np.stack([res.results[b]["out"].reshape(T, OUT) for b in range(B)])
    return out.astype(np.float32)


def kernel_traced(**inputs):
    """same as kernel() but returns (output, BassKernelResults) with timing"""
    nc = _get_nc()
    res = run_bass_kernel_spmd(nc, _in_maps(inputs), list(range(B)), trace=True)
    out = np.stack([res.results[b]["out"].reshape(T, OUT) for b in range(B)])
    return out.astype(np.float32), res
